# revision 7
# baseline (speedup 1.0000x reference)
"""HGT-style GNN message passing on 8 TRN2 NeuronCores.

Strategy (per sharding hint): partition nodes + incoming edges by dst across
8 cores. Each core:
  A) projects its 12500 nodes' features (q, and fused k/v projections) on
     TensorE (node-major, via host-transposed h), writing an interleaved
     [k|v] bf16 block to DRAM,
  B) AllGathers the full 100k-node k|v table,
  C) per 128-node tile with degree-padded neighbor lists: batched indirect
     DMA gathers of k|v rows per edge slot, DVE dot+softmax (padding slots
     point at a zero table row -> es=1, v=0; corrected by subtracting the
     pad count from the softmax denominator), weighted aggregation, and the
     output projection on TensorE.
Host side does only index/layout prep (degree bucketing, edge slot tables,
inverse permutation) and the final unshard.
"""

import sys
import types
import numpy as np
from contextlib import ExitStack

import ml_dtypes

BF16 = ml_dtypes.bfloat16

N = 100000
E = 1600000
IN = 256
OUT = 128
H = 8
DK = 16
NCORES = 8
NL = N // NCORES           # 12500 nodes per core
P = 128
NTILES = (NL + P - 1) // P  # 98
SLOTS = NTILES * P          # 12544 (with ghosts)
BLK = SLOTS + 4             # 12548 rows per rank block (last 4 zero)
NCHUNK = 4                  # all-gather chunks (overlap with phase A)
CROWS = BLK // NCHUNK       # 3137 rows per chunk per rank
ZROW = SLOTS                # zero-row bucket position within a block
TBL = BLK * NCORES


def _prep(h, Wq, bq, Wk, bk, Wv, bv, Wmsg, bmsg, Wattn, battn, Wa, ba, src, dst):
    h = np.asarray(h, np.float32)
    src = np.asarray(src).astype(np.int64)
    dst = np.asarray(dst).astype(np.int64)
    f32 = lambda x: np.asarray(x, np.float32)
    Wq, bq, Wa, ba = f32(Wq), f32(bq), f32(Wa), f32(ba)
    Wke = f32(Wk) @ f32(Wattn)
    bke = f32(bk) @ f32(Wattn) + f32(battn)
    Wve = f32(Wv) @ f32(Wmsg)
    bve = f32(bv) @ f32(Wmsg) + f32(bmsg)

    deg = np.bincount(dst, minlength=N)
    # per-core bucketed node order (degree desc, stable)
    orders = []       # local node ids in bucket order, per core
    for c in range(NCORES):
        d = deg[c * NL:(c + 1) * NL]
        o = np.argsort(-d, kind="stable")
        orders.append(o)
    # bucket position of each node within its core block
    bpos = np.empty(N, np.int64)
    for c in range(NCORES):
        bpos[c * NL + orders[c]] = np.arange(NL)

    # shared per-tile L schedule = max degree in tile across cores (>=1)
    Ls = np.zeros(NTILES, np.int64)
    for c in range(NCORES):
        d = deg[c * NL + orders[c]]
        d = np.concatenate([d, np.zeros(SLOTS - NL, np.int64)])
        Ls = np.maximum(Ls, d.reshape(NTILES, P).max(1))
    Ls = np.maximum(Ls, 1)
    offs = np.concatenate([[0], np.cumsum(Ls * P)]).astype(np.int64)
    TOT = int(offs[-1])

    # CSR of edges by dst
    order_e = np.argsort(dst, kind="stable")
    src_s = src[order_e]
    row_off = np.concatenate([[0], np.cumsum(deg)]).astype(np.int64)

    # global kv-table row per source node (bucketed position within block)
    bp = bpos[src_s]
    tbl_row = ((bp // CROWS) * (NCORES * CROWS)
               + (src_s // NL) * CROWS + (bp % CROWS))

    srcidxs, npads, hTs = [], [], []
    for c in range(NCORES):
        o = orders[c]
        glob = c * NL + o                                  # [NL]
        zrow_g = (ZROW // CROWS) * (NCORES * CROWS) + (ZROW % CROWS)
        si = np.full(TOT, zrow_g, np.int32)                # pad -> zero row
        npad = np.zeros(SLOTS, np.float32)
        dgs = deg[glob]
        for t in range(NTILES):
            L = int(Ls[t])
            blk = np.full((P, L), zrow_g, np.int32)
            for r in range(P):
                p = t * P + r
                if p >= NL:
                    npad[p] = L - 1
                    continue
                g = glob[p]
                d = int(dgs[p])
                if d > 0:
                    blk[r, :d] = tbl_row[row_off[g]:row_off[g] + d]
                    npad[p] = L - d
                else:
                    npad[p] = L - 1
            si[offs[t]:offs[t + 1]] = blk.reshape(-1)
        srcidxs.append(si)
        npads.append(npad)
        hT = h[glob].T                                     # [256, NL]
        hT = np.concatenate([hT, np.repeat(hT[:, :1], SLOTS - NL, 1)], 1)
        hTs.append(np.ascontiguousarray(hT.astype(BF16)))

    w = dict(
        wq=Wq.astype(BF16), wke=Wke.astype(BF16), wve=Wve.astype(BF16),
        bq=bq.reshape(1, OUT).astype(BF16), bke=bke.reshape(1, OUT).astype(BF16),
        bve=bve.reshape(1, OUT).astype(BF16),
        wa=Wa.astype(BF16), ba=ba.reshape(1, OUT).astype(BF16),
    )
    return w, hTs, srcidxs, npads, orders, Ls, offs, TOT


def _build(Ls, TOT):
    from concourse import bass, mybir, tile, bacc
    from concourse.masks import make_identity

    f32, bf16, i32 = mybir.dt.float32, mybir.dt.bfloat16, mybir.dt.int32
    nc = bacc.Bacc("TRN2", target_bir_lowering=False, debug=False,
                   enable_asserts=True, num_devices=NCORES)
    hT = nc.dram_tensor("hT", [IN, SLOTS], bf16, kind="ExternalInput")
    wq = nc.dram_tensor("wq", [IN, OUT], bf16, kind="ExternalInput")
    wke = nc.dram_tensor("wke", [IN, OUT], bf16, kind="ExternalInput")
    wve = nc.dram_tensor("wve", [IN, OUT], bf16, kind="ExternalInput")
    bq = nc.dram_tensor("bq", [1, OUT], bf16, kind="ExternalInput")
    bke = nc.dram_tensor("bke", [1, OUT], bf16, kind="ExternalInput")
    bve = nc.dram_tensor("bve", [1, OUT], bf16, kind="ExternalInput")
    wa = nc.dram_tensor("wa", [OUT, OUT], bf16, kind="ExternalInput")
    ba = nc.dram_tensor("ba", [1, OUT], bf16, kind="ExternalInput")
    srcidx = nc.dram_tensor("srcidx", [TOT, 1], i32, kind="ExternalInput")
    npadf = nc.dram_tensor("npadf", [SLOTS, 1], f32, kind="ExternalInput")
    out = nc.dram_tensor("out", [SLOTS, OUT], f32, kind="ExternalOutput")
    kv_loc = [nc.dram_tensor(f"kv_loc{i}", [CROWS, 2 * OUT], bf16,
                             kind="Internal") for i in range(NCHUNK)]
    kv_tbl = nc.dram_tensor("kv_tbl", [TBL, 2 * OUT], bf16,
                            kind="Internal", addr_space="Shared")

    def write_rows(src_tile, row0):
        # DMA [P, 2*OUT] sbuf tile rows into the chunked local blocks
        done = 0
        while done < P:
            r = row0 + done
            ci = r // CROWS
            lo = r % CROWS
            n = min(P - done, CROWS - lo)
            nc.sync.dma_start(kv_loc[ci][lo:lo + n, :],
                              src_tile[done:done + n, :])
            done += n

    offs = np.concatenate([[0], np.cumsum(np.asarray(Ls) * P)]).astype(np.int64)

    with tile.TileContext(nc) as tc:
        with ExitStack() as ctx:
            const = ctx.enter_context(tc.tile_pool(name="const", bufs=1))
            sb = ctx.enter_context(tc.tile_pool(name="sb", bufs=3))
            big = ctx.enter_context(tc.tile_pool(name="big", bufs=2))
            ps = ctx.enter_context(tc.tile_pool(name="ps", bufs=2, space="PSUM"))

            ident = const.tile([P, P], f32)
            make_identity(nc, ident[:])
            ones = const.tile([1, P], bf16)
            nc.gpsimd.memset(ones[:], 1.0)
            # replicated weights resident in SBUF
            wq_t = const.tile([P, (IN // P) * OUT], bf16)
            wke_t = const.tile([P, (IN // P) * OUT], bf16)
            wve_t = const.tile([P, (IN // P) * OUT], bf16)
            for wt, wd in ((wq_t, wq), (wke_t, wke), (wve_t, wve)):
                for ch in range(IN // P):
                    nc.sync.dma_start(wt[:, ch * OUT:(ch + 1) * OUT],
                                      wd[ch * P:(ch + 1) * P, :])
            wa_t = const.tile([P, OUT], bf16)
            nc.sync.dma_start(wa_t[:], wa[:])
            b_t = {}
            for nm, bd in (("bq", bq), ("bke", bke), ("bve", bve), ("ba", ba)):
                b_t[nm] = const.tile([1, OUT], bf16, tag=f"b_{nm}",
                                     name=f"b_{nm}")
                nc.sync.dma_start(b_t[nm][:], bd[:])
            # q stays resident in SBUF for the whole edge phase
            q_all = const.tile([P, NTILES * OUT], bf16)
            zrow = const.tile([4, 2 * OUT], bf16)
            nc.gpsimd.memset(zrow[:], 0.0)
            zl = SLOTS % CROWS
            nc.sync.dma_start(kv_loc[NCHUNK - 1][zl:zl + 4, :], zrow[:])

            # ---- phase A: projections ----
            for t in range(NTILES):
                hh = sb.tile([P, (IN // P) * P], bf16, tag="hh")
                for ch in range(IN // P):
                    nc.sync.dma_start(
                        hh[:, ch * P:(ch + 1) * P],
                        hT[ch * P:(ch + 1) * P, t * P:(t + 1) * P])
                kv_sb = sb.tile([P, 2 * OUT], bf16, tag="kv_sb")
                for wt, bn, dst_ap in (
                    (wq_t, "bq", q_all[:, t * OUT:(t + 1) * OUT]),
                    (wke_t, "bke", kv_sb[:, 0:OUT]),
                    (wve_t, "bve", kv_sb[:, OUT:2 * OUT]),
                ):
                    pj = ps.tile([P, OUT], f32, tag="proj", space="PSUM")
                    nc.tensor.matmul(out=pj[:], lhsT=hh[:, 0:P],
                                     rhs=wt[:, 0:OUT],
                                     start=True, stop=False)
                    nc.tensor.matmul(out=pj[:], lhsT=hh[:, P:2 * P],
                                     rhs=wt[:, OUT:2 * OUT],
                                     start=False, stop=False)
                    nc.tensor.matmul(out=pj[:], lhsT=ones[:], rhs=b_t[bn][:],
                                     start=False, stop=True)
                    nc.scalar.activation(dst_ap, pj[:],
                                         mybir.ActivationFunctionType.Copy)
                write_rows(kv_sb, t * P)

            # ---- phase B: chunked all-gathers (overlap phase A) ----
            CS = NCORES * CROWS
            for i in range(NCHUNK):
                nc.gpsimd.collective_compute(
                    "AllGather", mybir.AluOpType.bypass,
                    replica_groups=[list(range(NCORES))],
                    ins=[kv_loc[i][:]], outs=[kv_tbl[i * CS:(i + 1) * CS, :]],
                )

            # ---- phase C: per-tile edge compute ----
            Lmax = int(max(Ls))
            for t in range(NTILES):
                L = int(Ls[t])
                o0 = int(offs[t])
                idx_t = sb.tile([P, Lmax], i32, tag="idx")
                nc.sync.dma_start(
                    idx_t[:, :L],
                    srcidx[o0:o0 + P * L, :].rearrange(
                        "(p j) one -> p (j one)", p=P))
                np_t = sb.tile([P, 1], f32, tag="npad")
                nc.sync.dma_start(np_t[:], npadf[t * P:(t + 1) * P, :])
                kvg = big.tile([P, Lmax * 2 * OUT], bf16, tag="kvg")
                for j in range(L):
                    nc.gpsimd.indirect_dma_start(
                        out=kvg[:, j * 2 * OUT:(j + 1) * 2 * OUT],
                        out_offset=None,
                        in_=kv_tbl[:],
                        in_offset=bass.IndirectOffsetOnAxis(
                            ap=idx_t[:, j:j + 1], axis=0),
                    )
                kvv = kvg[:, :L * 2 * OUT].rearrange(
                    "p (j f) -> p j f", j=L)
                q_t = q_all[:, t * OUT:(t + 1) * OUT]
                prod = big.tile([P, Lmax * OUT], f32, tag="prod")
                nc.vector.tensor_tensor(
                    out=prod[:, :L * OUT].rearrange("p (j f) -> p j f", j=L),
                    in0=kvv[:, :, 0:OUT],
                    in1=q_t.rearrange("p (one f) -> p one f", one=1
                                      ).to_broadcast([P, L, OUT]),
                    op=mybir.AluOpType.mult)
                s_t = sb.tile([P, Lmax * H], f32, tag="s")
                nc.vector.tensor_reduce(
                    out=s_t[:, :L * H].rearrange("p (j h) -> p j h", j=L),
                    in_=prod[:, :L * OUT].rearrange(
                        "p (j h d) -> p j h d", j=L, h=H),
                    axis=mybir.AxisListType.X, op=mybir.AluOpType.add)
                es = sb.tile([P, Lmax * H], bf16, tag="es")
                nc.scalar.activation(es[:, :L * H], s_t[:, :L * H],
                                     mybir.ActivationFunctionType.Exp,
                                     scale=1.0 / np.sqrt(DK))
                wv = big.tile([P, Lmax * OUT], bf16, tag="wv")
                nc.vector.tensor_tensor(
                    out=wv[:, :L * OUT].rearrange(
                        "p (j h d) -> p j h d", j=L, h=H),
                    in0=kvv[:, :, OUT:2 * OUT].rearrange(
                        "p j (h d) -> p j h d", h=H),
                    in1=es[:, :L * H].rearrange(
                        "p (j h one) -> p j h one", j=L, one=1
                        ).to_broadcast([P, L, H, DK]),
                    op=mybir.AluOpType.mult)
                z = sb.tile([P, H], f32, tag="z")
                nc.vector.tensor_reduce(
                    out=z[:],
                    in_=es[:, :L * H].rearrange("p (j h) -> p h j", j=L),
                    axis=mybir.AxisListType.X, op=mybir.AluOpType.add)
                z2 = sb.tile([P, H], f32, tag="z2")
                nc.vector.tensor_scalar_sub(z2[:], z[:], np_t[:, :1])
                zr = sb.tile([P, H], f32, tag="zr")
                nc.vector.reciprocal(zr[:], z2[:])
                agg = sb.tile([P, OUT], f32, tag="agg")
                nc.vector.tensor_reduce(
                    out=agg[:],
                    in_=wv[:, :L * OUT].rearrange("p (j f) -> p f j", j=L),
                    axis=mybir.AxisListType.X, op=mybir.AluOpType.add)
                aggn = sb.tile([P, OUT], f32, tag="aggn")
                nc.vector.tensor_tensor(
                    out=aggn[:].rearrange("p (h d) -> p h d", h=H),
                    in0=agg[:].rearrange("p (h d) -> p h d", h=H),
                    in1=zr[:].rearrange("p (h one) -> p h one", one=1
                                        ).to_broadcast([P, H, DK]),
                    op=mybir.AluOpType.mult)
                tp = ps.tile([P, P], f32, tag="tp", space="PSUM")
                nc.tensor.transpose(out=tp[:], in_=aggn[:], identity=ident[:])
                aggT = sb.tile([P, P], bf16, tag="aggT")
                nc.scalar.activation(aggT[:], tp[:],
                                     mybir.ActivationFunctionType.Copy)
                op_ = ps.tile([P, OUT], f32, tag="op", space="PSUM")
                nc.tensor.matmul(out=op_[:], lhsT=aggT[:], rhs=wa_t[:],
                                 start=True, stop=False)
                nc.tensor.matmul(out=op_[:], lhsT=ones[:], rhs=b_t["ba"][:],
                                 start=False, stop=True)
                ot = sb.tile([P, OUT], f32, tag="ot")
                nc.scalar.activation(ot[:], op_[:],
                                     mybir.ActivationFunctionType.Copy)
                nc.sync.dma_start(out[t * P:(t + 1) * P, :], ot[:])

    nc.compile()
    return nc


def kernel(h, Wq, bq, Wk, bk, Wv, bv, Wmsg, bmsg, Wattn, battn, Wa, ba,
           src, dst, _profile=[None]):
    from concourse.bass_utils import run_bass_kernel_spmd

    w, hTs, srcidxs, npads, orders, Ls, offs, TOT = _prep(
        h, Wq, bq, Wk, bk, Wv, bv, Wmsg, bmsg, Wattn, battn, Wa, ba, src, dst)
    nc = _build(Ls, TOT)
    in_maps = []
    for c in range(NCORES):
        m = dict(w)
        m["hT"] = hTs[c]
        m["srcidx"] = srcidxs[c].reshape(TOT, 1)
        m["npadf"] = npads[c].reshape(SLOTS, 1)
        in_maps.append(m)
    trace = _profile[0] is not None
    res = run_bass_kernel_spmd(nc, in_maps, core_ids=list(range(NCORES)),
                               trace=trace)
    if trace:
        _profile[0] = res.exec_time_ns
    full = np.empty((N, OUT), np.float32)
    for c in range(NCORES):
        oc = np.asarray(res.results[c]["out"], np.float32)
        full[c * NL + orders[c]] = oc[:NL]
    return full


# revision 8
# speedup vs baseline: 1.0401x; 1.0401x over previous
"""HGT-style GNN message passing on 8 TRN2 NeuronCores.

Strategy (per sharding hint): partition nodes + incoming edges by dst across
8 cores. Each core:
  A) projects its 12500 nodes' features (q, and fused k/v projections) on
     TensorE (node-major, via host-transposed h), writing an interleaved
     [k|v] bf16 block to DRAM,
  B) AllGathers the full 100k-node k|v table,
  C) per 128-node tile with degree-padded neighbor lists: batched indirect
     DMA gathers of k|v rows per edge slot, DVE dot+softmax (padding slots
     point at a zero table row -> es=1, v=0; corrected by subtracting the
     pad count from the softmax denominator), weighted aggregation, and the
     output projection on TensorE.
Host side does only index/layout prep (degree bucketing, edge slot tables,
inverse permutation) and the final unshard.
"""

import sys
import types
import numpy as np
from contextlib import ExitStack

import ml_dtypes

BF16 = ml_dtypes.bfloat16

N = 100000
E = 1600000
IN = 256
OUT = 128
H = 8
DK = 16
NCORES = 8
NL = N // NCORES           # 12500 nodes per core
P = 128
NTILES = (NL + P - 1) // P  # 98
SLOTS = NTILES * P          # 12544 (with ghosts)
BLK = SLOTS + 4             # 12548 rows per rank block (last 4 zero)
NCHUNK = 4                  # all-gather chunks (overlap with phase A)
CROWS = BLK // NCHUNK       # 3137 rows per chunk per rank
ZROW = SLOTS                # zero-row bucket position within a block
TBL = BLK * NCORES


def _prep(h, Wq, bq, Wk, bk, Wv, bv, Wmsg, bmsg, Wattn, battn, Wa, ba, src, dst):
    h = np.asarray(h, np.float32)
    src = np.asarray(src).astype(np.int64)
    dst = np.asarray(dst).astype(np.int64)
    f32 = lambda x: np.asarray(x, np.float32)
    Wq, bq, Wa, ba = f32(Wq), f32(bq), f32(Wa), f32(ba)
    Wke = f32(Wk) @ f32(Wattn)
    bke = f32(bk) @ f32(Wattn) + f32(battn)
    Wve = f32(Wv) @ f32(Wmsg)
    bve = f32(bv) @ f32(Wmsg) + f32(bmsg)

    deg = np.bincount(dst, minlength=N)
    # per-core bucketed node order (degree desc, stable)
    orders = []       # local node ids in bucket order, per core
    for c in range(NCORES):
        d = deg[c * NL:(c + 1) * NL]
        o = np.argsort(-d, kind="stable")
        orders.append(o)
    # bucket position of each node within its core block
    bpos = np.empty(N, np.int64)
    for c in range(NCORES):
        bpos[c * NL + orders[c]] = np.arange(NL)

    # shared per-tile L schedule = max degree in tile across cores (>=1)
    Ls = np.zeros(NTILES, np.int64)
    for c in range(NCORES):
        d = deg[c * NL + orders[c]]
        d = np.concatenate([d, np.zeros(SLOTS - NL, np.int64)])
        Ls = np.maximum(Ls, d.reshape(NTILES, P).max(1))
    Ls = np.maximum(Ls, 1)
    offs = np.concatenate([[0], np.cumsum(Ls * P)]).astype(np.int64)
    TOT = int(offs[-1])

    # CSR of edges by dst
    order_e = np.argsort(dst, kind="stable")
    src_s = src[order_e]
    row_off = np.concatenate([[0], np.cumsum(deg)]).astype(np.int64)

    # global kv-table row per source node (bucketed position within block)
    bp = bpos[src_s]
    tbl_row = ((bp // CROWS) * (NCORES * CROWS)
               + (src_s // NL) * CROWS + (bp % CROWS))

    srcidxs, npads, hTs = [], [], []
    for c in range(NCORES):
        o = orders[c]
        glob = c * NL + o                                  # [NL]
        zrow_g = (ZROW // CROWS) * (NCORES * CROWS) + (ZROW % CROWS)
        si = np.full(TOT, zrow_g, np.int32)                # pad -> zero row
        npad = np.zeros(SLOTS, np.float32)
        dgs = deg[glob]
        for t in range(NTILES):
            L = int(Ls[t])
            blk = np.full((P, L), zrow_g, np.int32)
            for r in range(P):
                p = t * P + r
                if p >= NL:
                    npad[p] = L - 1
                    continue
                g = glob[p]
                d = int(dgs[p])
                if d > 0:
                    blk[r, :d] = tbl_row[row_off[g]:row_off[g] + d]
                    npad[p] = L - d
                else:
                    npad[p] = L - 1
            si[offs[t]:offs[t + 1]] = blk.reshape(-1)
        srcidxs.append(si)
        npads.append(npad)
        hT = h[glob].T                                     # [256, NL]
        hT = np.concatenate([hT, np.repeat(hT[:, :1], SLOTS - NL, 1)], 1)
        hTs.append(np.ascontiguousarray(hT.astype(BF16)))

    w = dict(
        wq=Wq.astype(BF16), wke=Wke.astype(BF16), wve=Wve.astype(BF16),
        bq=bq.reshape(1, OUT).astype(BF16), bke=bke.reshape(1, OUT).astype(BF16),
        bve=bve.reshape(1, OUT).astype(BF16),
        wa=Wa.astype(BF16), ba=ba.reshape(1, OUT).astype(BF16),
    )
    return w, hTs, srcidxs, npads, orders, Ls, offs, TOT


def _build(Ls, TOT):
    from concourse import bass, mybir, tile, bacc
    from concourse.masks import make_identity

    f32, bf16, i32 = mybir.dt.float32, mybir.dt.bfloat16, mybir.dt.int32
    nc = bacc.Bacc("TRN2", target_bir_lowering=False, debug=False,
                   enable_asserts=True, num_devices=NCORES)
    hT = nc.dram_tensor("hT", [IN, SLOTS], bf16, kind="ExternalInput")
    wq = nc.dram_tensor("wq", [IN, OUT], bf16, kind="ExternalInput")
    wke = nc.dram_tensor("wke", [IN, OUT], bf16, kind="ExternalInput")
    wve = nc.dram_tensor("wve", [IN, OUT], bf16, kind="ExternalInput")
    bq = nc.dram_tensor("bq", [1, OUT], bf16, kind="ExternalInput")
    bke = nc.dram_tensor("bke", [1, OUT], bf16, kind="ExternalInput")
    bve = nc.dram_tensor("bve", [1, OUT], bf16, kind="ExternalInput")
    wa = nc.dram_tensor("wa", [OUT, OUT], bf16, kind="ExternalInput")
    ba = nc.dram_tensor("ba", [1, OUT], bf16, kind="ExternalInput")
    srcidx = nc.dram_tensor("srcidx", [TOT, 1], i32, kind="ExternalInput")
    npadf = nc.dram_tensor("npadf", [SLOTS, 1], f32, kind="ExternalInput")
    out = nc.dram_tensor("out", [SLOTS, OUT], f32, kind="ExternalOutput")
    kv_loc = [nc.dram_tensor(f"kv_loc{i}", [CROWS, 2 * OUT], bf16,
                             kind="Internal") for i in range(NCHUNK)]
    kv_tbl = nc.dram_tensor("kv_tbl", [TBL, 2 * OUT], bf16,
                            kind="Internal", addr_space="Shared")

    def write_rows(src_tile, row0):
        # DMA [P, 2*OUT] sbuf tile rows into the chunked local blocks
        done = 0
        while done < P:
            r = row0 + done
            ci = r // CROWS
            lo = r % CROWS
            n = min(P - done, CROWS - lo)
            nc.sync.dma_start(kv_loc[ci][lo:lo + n, :],
                              src_tile[done:done + n, :])
            done += n

    offs = np.concatenate([[0], np.cumsum(np.asarray(Ls) * P)]).astype(np.int64)

    with tile.TileContext(nc) as tc:
        with ExitStack() as ctx:
            const = ctx.enter_context(tc.tile_pool(name="const", bufs=1))
            sb = ctx.enter_context(tc.tile_pool(name="sb", bufs=3))
            big = ctx.enter_context(tc.tile_pool(name="big", bufs=2))
            ps = ctx.enter_context(tc.tile_pool(name="ps", bufs=2, space="PSUM"))

            ident = const.tile([P, P], f32)
            make_identity(nc, ident[:])
            ones = const.tile([1, P], bf16)
            nc.gpsimd.memset(ones[:], 1.0)
            # replicated weights resident in SBUF
            wq_t = const.tile([P, (IN // P) * OUT], bf16)
            wke_t = const.tile([P, (IN // P) * OUT], bf16)
            wve_t = const.tile([P, (IN // P) * OUT], bf16)
            for wt, wd in ((wq_t, wq), (wke_t, wke), (wve_t, wve)):
                for ch in range(IN // P):
                    nc.sync.dma_start(wt[:, ch * OUT:(ch + 1) * OUT],
                                      wd[ch * P:(ch + 1) * P, :])
            wa_t = const.tile([P, OUT], bf16)
            nc.sync.dma_start(wa_t[:], wa[:])
            b_t = {}
            for nm, bd in (("bq", bq), ("bke", bke), ("bve", bve), ("ba", ba)):
                b_t[nm] = const.tile([1, OUT], bf16, tag=f"b_{nm}",
                                     name=f"b_{nm}")
                nc.sync.dma_start(b_t[nm][:], bd[:])
            # q stays resident in SBUF for the whole edge phase
            q_all = const.tile([P, NTILES * OUT], bf16)
            zrow = const.tile([4, 2 * OUT], bf16)
            nc.gpsimd.memset(zrow[:], 0.0)
            zl = SLOTS % CROWS
            nc.sync.dma_start(kv_loc[NCHUNK - 1][zl:zl + 4, :], zrow[:])

            # ---- phase A: projections ----
            for t in range(NTILES):
                hh = sb.tile([P, (IN // P) * P], bf16, tag="hh")
                for ch in range(IN // P):
                    nc.sync.dma_start(
                        hh[:, ch * P:(ch + 1) * P],
                        hT[ch * P:(ch + 1) * P, t * P:(t + 1) * P])
                kv_sb = sb.tile([P, 2 * OUT], bf16, tag="kv_sb")
                for wt, bn, dst_ap in (
                    (wq_t, "bq", q_all[:, t * OUT:(t + 1) * OUT]),
                    (wke_t, "bke", kv_sb[:, 0:OUT]),
                    (wve_t, "bve", kv_sb[:, OUT:2 * OUT]),
                ):
                    pj = ps.tile([P, OUT], f32, tag="proj", space="PSUM")
                    nc.tensor.matmul(out=pj[:], lhsT=hh[:, 0:P],
                                     rhs=wt[:, 0:OUT],
                                     start=True, stop=False)
                    nc.tensor.matmul(out=pj[:], lhsT=hh[:, P:2 * P],
                                     rhs=wt[:, OUT:2 * OUT],
                                     start=False, stop=False)
                    nc.tensor.matmul(out=pj[:], lhsT=ones[:], rhs=b_t[bn][:],
                                     start=False, stop=True)
                    nc.scalar.activation(dst_ap, pj[:],
                                         mybir.ActivationFunctionType.Copy)
                write_rows(kv_sb, t * P)

            # ---- phase B: chunked all-gathers (overlap phase A) ----
            CS = NCORES * CROWS
            for i in range(NCHUNK):
                nc.gpsimd.collective_compute(
                    "AllGather", mybir.AluOpType.bypass,
                    replica_groups=[list(range(NCORES))],
                    ins=[kv_loc[i][:]], outs=[kv_tbl[i * CS:(i + 1) * CS, :]],
                )

            # ---- phase C: per-tile edge compute ----
            Lmax = int(max(Ls))
            for t in range(NTILES):
                L = int(Ls[t])
                o0 = int(offs[t])
                idx_t = sb.tile([P, Lmax], i32, tag="idx")
                nc.sync.dma_start(
                    idx_t[:, :L],
                    srcidx[o0:o0 + P * L, :].rearrange(
                        "(p j) one -> p (j one)", p=P))
                np_t = sb.tile([P, 1], f32, tag="npad")
                nc.sync.dma_start(np_t[:], npadf[t * P:(t + 1) * P, :])
                kvg = big.tile([P, Lmax * 2 * OUT], bf16, tag="kvg", bufs=3)
                for j in range(L):
                    nc.gpsimd.indirect_dma_start(
                        out=kvg[:, j * 2 * OUT:(j + 1) * 2 * OUT],
                        out_offset=None,
                        in_=kv_tbl[:],
                        in_offset=bass.IndirectOffsetOnAxis(
                            ap=idx_t[:, j:j + 1], axis=0),
                    )
                kvv = kvg[:, :L * 2 * OUT].rearrange(
                    "p (j f) -> p j f", j=L)
                q_t = q_all[:, t * OUT:(t + 1) * OUT]
                prod = big.tile([P, Lmax * OUT], bf16, tag="prod")
                nc.vector.tensor_tensor(
                    out=prod[:, :L * OUT].rearrange("p (j f) -> p j f", j=L),
                    in0=kvv[:, :, 0:OUT],
                    in1=q_t.rearrange("p (one f) -> p one f", one=1
                                      ).to_broadcast([P, L, OUT]),
                    op=mybir.AluOpType.mult)
                s_t = sb.tile([P, Lmax * H], f32, tag="s")
                nc.vector.tensor_reduce(
                    out=s_t[:, :L * H].rearrange("p (j h) -> p j h", j=L),
                    in_=prod[:, :L * OUT].rearrange(
                        "p (j h d) -> p j h d", j=L, h=H),
                    axis=mybir.AxisListType.X, op=mybir.AluOpType.add)
                es = sb.tile([P, Lmax * H], bf16, tag="es")
                nc.scalar.activation(es[:, :L * H], s_t[:, :L * H],
                                     mybir.ActivationFunctionType.Exp,
                                     scale=1.0 / np.sqrt(DK))
                wv = big.tile([P, Lmax * OUT], bf16, tag="wv")
                nc.vector.tensor_tensor(
                    out=wv[:, :L * OUT].rearrange(
                        "p (j h d) -> p j h d", j=L, h=H),
                    in0=kvv[:, :, OUT:2 * OUT].rearrange(
                        "p j (h d) -> p j h d", h=H),
                    in1=es[:, :L * H].rearrange(
                        "p (j h one) -> p j h one", j=L, one=1
                        ).to_broadcast([P, L, H, DK]),
                    op=mybir.AluOpType.mult)
                z = sb.tile([P, H], f32, tag="z")
                nc.vector.tensor_reduce(
                    out=z[:],
                    in_=es[:, :L * H].rearrange("p (j h) -> p h j", j=L),
                    axis=mybir.AxisListType.X, op=mybir.AluOpType.add)
                z2 = sb.tile([P, H], f32, tag="z2")
                nc.vector.tensor_scalar_sub(z2[:], z[:], np_t[:, :1])
                zr = sb.tile([P, H], f32, tag="zr")
                nc.vector.reciprocal(zr[:], z2[:])
                agg = sb.tile([P, OUT], f32, tag="agg")
                nc.vector.tensor_reduce(
                    out=agg[:],
                    in_=wv[:, :L * OUT].rearrange("p (j f) -> p f j", j=L),
                    axis=mybir.AxisListType.X, op=mybir.AluOpType.add)
                aggn = sb.tile([P, OUT], f32, tag="aggn")
                nc.vector.tensor_tensor(
                    out=aggn[:].rearrange("p (h d) -> p h d", h=H),
                    in0=agg[:].rearrange("p (h d) -> p h d", h=H),
                    in1=zr[:].rearrange("p (h one) -> p h one", one=1
                                        ).to_broadcast([P, H, DK]),
                    op=mybir.AluOpType.mult)
                tp = ps.tile([P, P], f32, tag="tp", space="PSUM")
                nc.tensor.transpose(out=tp[:], in_=aggn[:], identity=ident[:])
                aggT = sb.tile([P, P], bf16, tag="aggT")
                nc.scalar.activation(aggT[:], tp[:],
                                     mybir.ActivationFunctionType.Copy)
                op_ = ps.tile([P, OUT], f32, tag="op", space="PSUM")
                nc.tensor.matmul(out=op_[:], lhsT=aggT[:], rhs=wa_t[:],
                                 start=True, stop=False)
                nc.tensor.matmul(out=op_[:], lhsT=ones[:], rhs=b_t["ba"][:],
                                 start=False, stop=True)
                ot = sb.tile([P, OUT], f32, tag="ot")
                nc.scalar.activation(ot[:], op_[:],
                                     mybir.ActivationFunctionType.Copy)
                nc.sync.dma_start(out[t * P:(t + 1) * P, :], ot[:])

    nc.compile()
    return nc


def kernel(h, Wq, bq, Wk, bk, Wv, bv, Wmsg, bmsg, Wattn, battn, Wa, ba,
           src, dst, _profile=[None]):
    from concourse.bass_utils import run_bass_kernel_spmd

    w, hTs, srcidxs, npads, orders, Ls, offs, TOT = _prep(
        h, Wq, bq, Wk, bk, Wv, bv, Wmsg, bmsg, Wattn, battn, Wa, ba, src, dst)
    nc = _build(Ls, TOT)
    in_maps = []
    for c in range(NCORES):
        m = dict(w)
        m["hT"] = hTs[c]
        m["srcidx"] = srcidxs[c].reshape(TOT, 1)
        m["npadf"] = npads[c].reshape(SLOTS, 1)
        in_maps.append(m)
    trace = _profile[0] is not None
    res = run_bass_kernel_spmd(nc, in_maps, core_ids=list(range(NCORES)),
                               trace=trace)
    if trace:
        _profile[0] = res.exec_time_ns
    full = np.empty((N, OUT), np.float32)
    for c in range(NCORES):
        oc = np.asarray(res.results[c]["out"], np.float32)
        full[c * NL + orders[c]] = oc[:NL]
    return full
